# revision 1
# baseline (speedup 1.0000x reference)
"""BiLSTM (reference nn_CharBiGRU) Trainium2 Bass kernel.

Strategy:
  - 8 cores, batch-sharded (8 rows each); each core runs BOTH directions'
    LSTM scans interleaved (fwd over x, bwd over a host-rotated x_proc).
  - Host precomputes the per-batch time rotations (gathers) from mask
    lengths; the map s = (L-1-t) mod T is an involution, used on both the
    input and output sides of the backward scan.
  - Per step, gates for both dirs are computed as G[b, j] accumulated in
    PSUM via col-tiled matmuls: 4 PE column strips (one per gate i,f,o,g),
    stationary = h.T / x_t.T (8 cols each), streaming Wh.T / Wi.T.
    Bias enters as a K=1 matmul of a ones row.
  - Cell math runs on ACT (sigmoid/tanh) + DVE/GPSIMD elementwise with
    both dirs packed (fwd at free 0:512, bwd 512:1024).
  - h is recycled into stationary layout via PE transposes (4 per step).
"""

import numpy as np

B, T, D, H = 64, 512, 512, 512
G4 = 4 * H
NCORES = 8
BL = B // NCORES

_CACHE = {}


def build_kernel(T_steps=T, dtype_mm="float32"):
    import concourse.bass as bass
    import concourse.bacc as bacc
    import concourse.mybir as mybir
    from concourse.tile import TileContext
    from concourse.masks import make_identity

    fp32 = mybir.dt.float32
    AF = mybir.ActivationFunctionType

    # Bacc (not plain Bass): its compile() pass splits multi-waits into
    # event-semaphore chains and moves matmul waits onto LDWEIGHTS —
    # without it walrus rejects 2-wait matmuls ("Too many sync wait").
    nc = bacc.Bacc()
    xtt = nc.declare_dram_parameter("xtt", [2, T_steps, D, BL], fp32, isOutput=False)
    wht = nc.declare_dram_parameter("wht", [2, 4, 128, G4], fp32, isOutput=False)
    wit = nc.declare_dram_parameter("wit", [2, 4, 128, G4], fp32, isOutput=False)
    brow = nc.declare_dram_parameter("brow", [2, G4], fp32, isOutput=False)
    h0t = nc.declare_dram_parameter("h0t", [D, BL], fp32, isOutput=False)
    c0 = nc.declare_dram_parameter("c0", [BL, H], fp32, isOutput=False)
    ys = nc.declare_dram_parameter("ys", [2, T_steps, BL, H], fp32, isOutput=True)

    with TileContext(nc) as tc:
        with (
            tc.tile_pool(name="const", bufs=1) as constp,
            tc.tile_pool(name="wpool", bufs=1) as wpool,
            tc.tile_pool(name="state", bufs=1) as statep,
            tc.tile_pool(name="work", bufs=2) as workp,
            tc.tile_pool(name="xin", bufs=4) as xinp,
            tc.tile_pool(name="gpsum", bufs=2, space="PSUM") as psump,
            tc.tile_pool(name="ptpsum", bufs=2, space="PSUM") as ptp,
        ):
            ident = constp.tile([8, 8], fp32)
            make_identity(nc, ident[:, :])
            ones = constp.tile([1, 32], fp32)
            nc.gpsimd.memset(ones[:, :], 1.0)
            biasT = constp.tile([1, 2 * G4], fp32)
            for d in range(2):
                nc.sync.dma_start(out=biasT[0:1, d * G4:(d + 1) * G4], in_=brow[d:d + 1, :])

            # Weights in SBUF: one tile per (d, k) chunk = one DMA producer
            # each, so consuming matmuls carry a single sync-wait.
            whk = [[wpool.tile([128, G4], fp32, tag=f"wh{d}{k}", name=f"wh{d}{k}") for k in range(4)]
                   for d in range(2)]
            wik = [[wpool.tile([128, G4], fp32, tag=f"wi{d}{k}", name=f"wi{d}{k}") for k in range(4)]
                   for d in range(2)]
            for d in range(2):
                for k in range(4):
                    nc.sync.dma_start(out=whk[d][k][:, :], in_=wht[d, k])
                    nc.sync.dma_start(out=wik[d][k][:, :], in_=wit[d, k])

            # State: hT free = 16*k + 8*d + b ; c at base partition 32
            # (pairs with f-gate rows 32:40 in DVE tensor_tensor ops, which
            # require both SBUF inputs at the same base partition)
            hT = statep.tile([128, 64], fp32, tag="hT")
            C40 = statep.tile([40, 1024], fp32, tag="C40")
            c = C40[32:40, :]
            for k in range(4):
                nc.sync.dma_start(out=hT[:, 16 * k:16 * k + 8], in_=h0t[128 * k:128 * (k + 1), :])
                nc.sync.dma_start(out=hT[:, 16 * k + 8:16 * k + 16], in_=h0t[128 * k:128 * (k + 1), :])
            nc.sync.dma_start(out=c[:, 0:H], in_=c0[:, :])
            nc.sync.dma_start(out=c[:, H:2 * H], in_=c0[:, :])

            for t in range(T_steps):
                # x_t stationary tiles: one tile + one DMA per (d, k)
                xtk = [[xinp.tile([128, 8], fp32, tag=f"xt{d}{k}", name=f"xt{d}{k}") for k in range(4)]
                       for d in range(2)]
                for d in range(2):
                    for k in range(4):
                        nc.sync.dma_start(
                            out=xtk[d][k][:, :],
                            in_=xtt[d, t, 128 * k:128 * (k + 1), :],
                        )

                # Gates: G[32s + b, 512*d + jj] ; strip s = gate (i,f,o,g)
                G = psump.tile([128, 1024], fp32, tag="G")
                for d in range(2):
                    for s in range(4):
                        out_ap = G[32 * s:32 * s + 8, 512 * d:512 * (d + 1)]
                        tp = (0, 32 * s)
                        jo = d * G4 + 512 * s
                        # bias matmul writes the FULL 32-row strip (start=True)
                        # so no PSUM row is left uninitialized for the ACT reads
                        nc.tensor.matmul(
                            G[32 * s:32 * s + 32, 512 * d:512 * (d + 1)],
                            ones[0:1, 0:32], biasT[0:1, jo:jo + 512],
                            start=True, stop=False, tile_position=tp, skip_group_check=True,
                        )
                        for k in range(4):
                            nc.tensor.matmul(
                                out_ap, xtk[d][k][:, :],
                                wik[d][k][:, 512 * s:512 * s + 512],
                                start=False, stop=False, tile_position=tp, skip_group_check=True,
                            )
                        for k in range(4):
                            ho = 16 * k + 8 * d
                            nc.tensor.matmul(
                                out_ap, hT[:, ho:ho + 8],
                                whk[d][k][:, 512 * s:512 * s + 512],
                                start=False, stop=(k == 3), tile_position=tp, skip_group_check=True,
                            )

                # Activations: rows 0:96 = i,f,o -> sigmoid (i@0:8, f@32:40,
                # o@64:72); g -> tanh remapped to base 0 so it can pair with i
                A = workp.tile([96, 1024], fp32, tag="A")
                nc.scalar.activation(A[:, :], G[0:96, :], AF.Sigmoid)
                TG = workp.tile([8, 1024], fp32, tag="TG")
                nc.scalar.activation(TG[:, :], G[96:104, :], AF.Tanh)

                T1 = workp.tile([8, 1024], fp32, tag="T1")
                T2 = workp.tile([8, 1024], fp32, tag="T2")
                nc.vector.tensor_mul(T1[:, :], A[0:8, :], TG[:, :])       # bases 0,0
                nc.vector.tensor_mul(T2[:, :], A[32:40, :], C40[32:40, :])  # 32,32
                nc.vector.tensor_add(C40[32:40, :], T1[:, :], T2[:, :])   # out base 32
                TC = workp.tile([72, 1024], fp32, tag="TC")
                nc.scalar.activation(TC[64:72, :], C40[32:40, :], AF.Tanh)

                # h: fwd and bwd in separate base-0 tiles
                h2f = workp.tile([8, 512], fp32, tag="h2f")
                h2b = workp.tile([8, 512], fp32, tag="h2b")
                nc.vector.tensor_mul(h2f[:, :], A[64:72, 0:H], TC[64:72, 0:H])
                nc.gpsimd.tensor_mul(h2b[:, :], A[64:72, H:2 * H], TC[64:72, H:2 * H])

                nc.sync.dma_start(out=ys[0, t], in_=h2f[:, :])
                nc.sync.dma_start(out=ys[1, t], in_=h2b[:, :])

                # Recycle h into stationary layout: PT[:, 16k + 8d + b]
                PT = ptp.tile([128, 64], fp32, tag="PT")
                for k in range(4):
                    nc.tensor.transpose(
                        PT[:, 16 * k:16 * k + 8], h2f[:, 128 * k:128 * (k + 1)],
                        ident[:, :],
                    )
                    nc.tensor.transpose(
                        PT[:, 16 * k + 8:16 * k + 16], h2b[:, 128 * k:128 * (k + 1)],
                        ident[:, :],
                    )
                nc.vector.tensor_copy(hT[:, :], PT[:, :])

    nc.finalize()
    return nc


def _host_prep(inputs_emb, mask, h0, c0, Wi_f, Wh_f, b_f, Wi_b, Wh_b, b_b):
    x = np.asarray(inputs_emb, dtype=np.float32)
    mask = np.asarray(mask, dtype=np.float32)
    lengths = mask.astype(np.int32).sum(axis=1)  # [B]
    t_idx = np.arange(T, dtype=np.int64)[None, :]
    P = (lengths[:, None].astype(np.int64) - 1 - t_idx) % T  # [B, T] involution
    x_proc = np.take_along_axis(x, P[:, :, None], axis=1)  # [B, T, D]

    # xtt[d, t, :, b] layouts per core
    xtt_f = x.transpose(1, 2, 0)       # [T, D, B]
    xtt_b = x_proc.transpose(1, 2, 0)  # [T, D, B]

    # device strip order is (i, f, o, g); reference weights are (i, f, g, o)
    PERM = [0, 1, 3, 2]

    def chunks(W):
        # W: [4H, K] -> permute gate blocks -> W.T chunks [4, 128, 4H]
        W = np.asarray(W, dtype=np.float32)
        Wp = W.reshape(4, H, -1)[PERM].reshape(G4, -1)
        Wt = np.ascontiguousarray(Wp.T)  # [K, 4H]
        return Wt.reshape(4, 128, G4)

    def pbias(b):
        return np.asarray(b, np.float32).reshape(4, H)[PERM].reshape(G4)

    wht = np.stack([chunks(Wh_f), chunks(Wh_b)])  # [2, 4, 128, 4H]
    wit = np.stack([chunks(Wi_f), chunks(Wi_b)])
    brow = np.stack([pbias(b_f), pbias(b_b)])
    h0 = np.asarray(h0, np.float32)
    c0 = np.asarray(c0, np.float32)

    in_maps = []
    for cidx in range(NCORES):
        sl = slice(cidx * BL, (cidx + 1) * BL)
        in_maps.append({
            "xtt": np.ascontiguousarray(
                np.stack([xtt_f[:, :, sl], xtt_b[:, :, sl]])),
            "wht": wht, "wit": wit, "brow": brow,
            "h0t": np.ascontiguousarray(h0[sl].T),
            "c0": np.ascontiguousarray(c0[sl]),
        })
    return in_maps, P


def _host_post(results, P):
    ys_f = np.concatenate([r["ys"][0].transpose(1, 0, 2) for r in results], 0)  # [B,T,H]
    ys_b = np.concatenate([r["ys"][1].transpose(1, 0, 2) for r in results], 0)
    out_b = np.take_along_axis(ys_b, P[:, :, None], axis=1)
    return np.concatenate([ys_f, out_b], axis=-1).astype(np.float32)


def kernel(**inputs):
    from concourse.bass_utils import run_bass_kernel_spmd
    in_maps, P = _host_prep(**inputs)
    if "nc" not in _CACHE:
        _CACHE["nc"] = build_kernel()
    nc = _CACHE["nc"]
    res = run_bass_kernel_spmd(nc, in_maps, list(range(NCORES)))
    return _host_post(res.results, P)



# revision 5
# speedup vs baseline: 1.5063x; 1.5063x over previous
"""BiLSTM (reference nn_CharBiGRU) Trainium2 Bass kernel, v2.

Strategy (8 cores, batch-sharded, BL=8 rows/core, both directions per core):

  Phase A (GEMM): Xi[d,t,b,:] = x_d[b,t,:] @ Wi_d.T (+bias) for ALL t as a
    dense fp32r matmul (full-rate streaming, ap>=256), accumulated over 4
    K-chunks of D in PSUM [128=(16t x 8b), 512-gate-strip], evacuated via a
    DVE add (bias broadcast) with bf16 cast, staged to an internal-DRAM Xi
    tensor [d, m, 128, 2048].

  Phase B (recurrence): per (t, dir) only the h @ Wh.T part runs on PE:
    - inject: Xi_t enters the PSUM accumulation as an identity-stationary
      matmul ([I8|0] K=8, M=32 writes the full 32-row strip so no PSUM rows
      are left uninitialized), one per gate strip, col-tiled (0,32s).
    - 4 K-chunk matmuls vs bf16 Wh.T streams per strip (N=512).
    - gates g pre-scaled x2 on host so ONE sigmoid ACT call covers all 4
      strips: tanh(g) = 2*sigmoid(2g)-1 fixed up on DVE.
    - cell math split across DVE/GPSIMD; c kept fp32, gate outputs bf16.
    - h recycled to stationary layout via 4 PE transposes + DVE copy.
  The two directions are independent chains; Tile's scheduler pipelines
  them across engines.
"""

import numpy as np

B, T, D, H = 64, 512, 512, 512
G4 = 4 * H
NCORES = 8
BL = B // NCORES

_CACHE = {}


def build_kernel(T_steps=T):
    import concourse.bass as bass
    import concourse.bacc as bacc
    import concourse.mybir as mybir
    from concourse.tile import TileContext

    fp32 = mybir.dt.float32
    f32r = mybir.dt.float32r
    bf16 = mybir.dt.bfloat16
    AF = mybir.ActivationFunctionType
    ALU = mybir.AluOpType

    NM = (T_steps * BL) // 128  # m-chunks of (16 t x 8 b) per direction

    nc = bacc.Bacc()
    xg = nc.declare_dram_parameter("xg", [2, 4, 128, T_steps * BL], f32r, isOutput=False)
    wit = nc.declare_dram_parameter("wit", [2, 4, 128, G4], f32r, isOutput=False)
    wht = nc.declare_dram_parameter("wht", [2, 4, 128, G4], bf16, isOutput=False)
    brow = nc.declare_dram_parameter("brow", [2, G4], fp32, isOutput=False)
    identp = nc.declare_dram_parameter("identp", [8, 32], bf16, isOutput=False)
    h0t = nc.declare_dram_parameter("h0t", [4, 128, BL], bf16, isOutput=False)
    c0 = nc.declare_dram_parameter("c0", [BL, H], fp32, isOutput=False)
    ys = nc.declare_dram_parameter("ys", [2, T_steps, BL, H], bf16, isOutput=True)

    with TileContext(nc) as tc:
        with (
            tc.tile_pool(name="persist", bufs=1) as pp,
            tc.tile_pool(name="dramp", bufs=1, space="DRAM") as dp,
        ):
            ident = pp.tile([8, 32], bf16, name="ident")
            nc.sync.dma_start(out=ident[:, :], in_=identp[:, :])

            # Wh.T chunks, one tile per (d, k) so consumers have 1 producer
            whk = [[pp.tile([128, G4], bf16, name=f"wh{d}{k}") for k in range(4)]
                   for d in range(2)]
            for d in range(2):
                for k in range(4):
                    nc.sync.dma_start(out=whk[d][k][:, :], in_=wht[d, k])

            # bias broadcast to 128 partitions (added during Xi evacuation)
            brow_sb = pp.tile([1, 2 * G4], fp32, name="brow_sb")
            nc.sync.dma_start(out=brow_sb[0:1, :], in_=brow[:, :])
            bias_bc = pp.tile([128, 2 * G4], fp32, name="bias_bc")
            nc.gpsimd.partition_broadcast(bias_bc[:, :], brow_sb[0:1, :])

            # recurrent state
            hT = [pp.tile([128, 4 * BL], bf16, name=f"hT{d}") for d in range(2)]
            Ct = [pp.tile([40, H], fp32, name=f"C{d}") for d in range(2)]
            for d in range(2):
                for k in range(4):
                    nc.sync.dma_start(out=hT[d][:, 8 * k:8 * k + 8], in_=h0t[k])
                nc.sync.dma_start(out=Ct[d][32:40, :], in_=c0[:, :])

            # internal-DRAM Xi staging (own pool so it isn't charged to SBUF)
            xi_dram = dp.tile([2, NM, 128, G4], bf16, name="xi_dram")

            # ---------------- Phase A: Xi GEMM ----------------
            with (
                tc.tile_pool(name="witp", bufs=1) as witp,
                tc.tile_pool(name="xkp", bufs=8) as xkp,
                tc.tile_pool(name="gemmps", bufs=4, space="PSUM") as gemmps,
                tc.tile_pool(name="stagep", bufs=3) as stagep,
            ):
                wit_sb = [[witp.tile([128, G4], f32r, name=f"wi{d}{k}") for k in range(4)]
                          for d in range(2)]
                for d in range(2):
                    for k in range(4):
                        nc.sync.dma_start(out=wit_sb[d][k][:, :], in_=wit[d, k])

                for d in range(2):
                    for m in range(NM):
                        xk = [xkp.tile([128, 128], f32r, tag=f"xk{k}", name=f"xk{d}{m}{k}")
                              for k in range(4)]
                        for k in range(4):
                            nc.sync.dma_start(
                                out=xk[k][:, :], in_=xg[d, k, :, 128 * m:128 * (m + 1)])
                        st = stagep.tile([128, G4], bf16, tag="st", name=f"st{d}{m}")
                        for s in range(4):
                            P = gemmps.tile([128, 512], fp32, tag="P", name=f"P{d}{m}{s}")
                            for k in range(4):
                                nc.tensor.matmul(
                                    P[:, :], xk[k][:, :],
                                    wit_sb[d][k][:, 512 * s:512 * (s + 1)],
                                    start=(k == 0), stop=(k == 3),
                                )
                            # evac + bias + bf16 cast
                            nc.vector.scalar_tensor_tensor(
                                out=st[:, 512 * s:512 * (s + 1)],
                                in0=P[:, :], scalar=0.0,
                                in1=bias_bc[:, d * G4 + 512 * s:d * G4 + 512 * (s + 1)],
                                op0=ALU.add, op1=ALU.add,
                            )
                        nc.sync.dma_start(out=xi_dram[d, m], in_=st[:, :])

            # ---------------- Phase B: recurrence ----------------
            with (
                tc.tile_pool(name="xip", bufs=6) as xip,
                tc.tile_pool(name="gps0", bufs=2, space="PSUM") as gps0,
                tc.tile_pool(name="gps1", bufs=2, space="PSUM") as gps1,
                tc.tile_pool(name="ptp0", bufs=1, space="PSUM") as ptp0,
                tc.tile_pool(name="ptp1", bufs=1, space="PSUM") as ptp1,
                tc.tile_pool(name="workp", bufs=2) as workp,
            ):
                gpools = [gps0, gps1]
                ptpools = [ptp0, ptp1]
                for t in range(T_steps):
                    for d in range(2):
                        xi_t = xip.tile([8, G4], bf16, tag=f"xi{d}", name=f"xi{d}_{t}")
                        mm, r = divmod(t, 16)
                        nc.sync.dma_start(
                            out=xi_t[:, :], in_=xi_dram[d, mm, 8 * r:8 * r + 8, :])

                        G = gpools[d].tile([128, 512], fp32, tag=f"G{d}", name=f"G{d}_{t}")
                        for s in range(4):
                            tp = (0, 32 * s)
                            out8 = G[32 * s:32 * s + 8, :]
                            nc.tensor.matmul(
                                G[32 * s:32 * s + 32, :], ident[0:8, 0:32],
                                xi_t[0:8, 512 * s:512 * (s + 1)],
                                start=True, stop=False, tile_position=tp,
                                skip_group_check=True,
                            )
                            for k in range(4):
                                nc.tensor.matmul(
                                    out8, hT[d][:, 8 * k:8 * k + 8],
                                    whk[d][k][:, 512 * s:512 * (s + 1)],
                                    start=False, stop=(k == 3), tile_position=tp,
                                    skip_group_check=True,
                                )

                        # all 4 strips are sigmoid (g-gate weights pre-scaled
                        # x2; tanh(g) = 2*sigmoid(2g) - 1 fixed up below)
                        Y = workp.tile([104, 512], bf16, tag=f"Y{d}", name=f"Y{d}_{t}")
                        nc.scalar.activation(Y[:, :], G[0:104, :], AF.Sigmoid)

                        TG = workp.tile([8, 512], bf16, tag=f"TG{d}", name=f"TG{d}_{t}")
                        nc.vector.tensor_scalar(
                            TG[:, :], Y[96:104, :], 2.0, -1.0, ALU.mult, ALU.add)
                        U = workp.tile([8, 512], bf16, tag=f"U{d}", name=f"U{d}_{t}")
                        nc.gpsimd.tensor_mul(U[:, :], Y[0:8, :], TG[:, :])
                        V = workp.tile([8, 512], fp32, tag=f"V{d}", name=f"V{d}_{t}")
                        nc.vector.tensor_mul(V[:, :], Y[32:40, :], Ct[d][32:40, :])
                        nc.vector.tensor_add(Ct[d][32:40, :], U[:, :], V[:, :])
                        Z = workp.tile([72, 512], bf16, tag=f"Z{d}", name=f"Z{d}_{t}")
                        nc.scalar.activation(Z[64:72, :], Ct[d][32:40, :], AF.Tanh)
                        ht = workp.tile([8, 512], bf16, tag=f"h{d}", name=f"h{d}_{t}")
                        nc.gpsimd.tensor_mul(ht[:, :], Y[64:72, :], Z[64:72, :])

                        nc.sync.dma_start(out=ys[d, t], in_=ht[:, :])

                        PT = ptpools[d].tile([128, 4 * BL], bf16, tag=f"PT{d}",
                                             name=f"PT{d}_{t}")
                        for k in range(4):
                            nc.tensor.transpose(
                                PT[:, 8 * k:8 * k + 8], ht[:, 128 * k:128 * (k + 1)],
                                ident[0:8, 0:8])
                        nc.vector.tensor_copy(hT[d][:, :], PT[:, :])

    nc.finalize()
    return nc


def _host_prep(inputs_emb, mask, h0, c0, Wi_f, Wh_f, b_f, Wi_b, Wh_b, b_b):
    import ml_dtypes
    bf16 = ml_dtypes.bfloat16

    x = np.asarray(inputs_emb, dtype=np.float32)
    mask = np.asarray(mask, dtype=np.float32)
    lengths = mask.astype(np.int32).sum(axis=1)  # [B]
    t_idx = np.arange(T, dtype=np.int64)[None, :]
    P = (lengths[:, None].astype(np.int64) - 1 - t_idx) % T  # [B, T] involution
    x_proc = np.take_along_axis(x, P[:, :, None], axis=1)  # [B, T, D]

    # device strip order is (i, f, o, g); reference weights are (i, f, g, o).
    # The g strip (device 3) is pre-scaled x2: tanh(g) = 2*sigmoid(2g) - 1.
    PERM = [0, 1, 3, 2]
    GSCALE = np.array([1.0, 1.0, 1.0, 2.0], np.float32)[:, None, None]

    def chunks(W, dt):
        W = np.asarray(W, dtype=np.float32)
        Wp = (W.reshape(4, H, -1)[PERM] * GSCALE).reshape(G4, -1)
        Wt = np.ascontiguousarray(Wp.T)  # [K, 4H]
        return Wt.reshape(4, 128, G4).astype(dt)

    def pbias(b):
        return (np.asarray(b, np.float32).reshape(4, H)[PERM]
                * GSCALE[:, :, 0]).reshape(G4)

    wit = np.stack([chunks(Wi_f, np.float32), chunks(Wi_b, np.float32)])
    wht = np.stack([chunks(Wh_f, bf16), chunks(Wh_b, bf16)])
    brow = np.stack([pbias(b_f), pbias(b_b)]).astype(np.float32)

    identp = np.zeros((8, 32), np.float32)
    identp[np.arange(8), np.arange(8)] = 1.0
    identp = identp.astype(bf16)

    h0 = np.asarray(h0, np.float32)
    c0 = np.asarray(c0, np.float32)

    in_maps = []
    for cidx in range(NCORES):
        sl = slice(cidx * BL, (cidx + 1) * BL)
        xf = x[sl]        # [BL, T, D]
        xb = x_proc[sl]   # [BL, T, D]
        # xg[d, k, p, t*BL+b] = x_d[b, t, 128k+p]
        xg = np.stack([
            np.ascontiguousarray(xf.transpose(2, 1, 0).reshape(4, 128, T * BL)),
            np.ascontiguousarray(xb.transpose(2, 1, 0).reshape(4, 128, T * BL)),
        ])
        h0c = h0[sl]  # [BL, H]
        h0t = np.ascontiguousarray(h0c.T.reshape(4, 128, BL)).astype(bf16)
        in_maps.append({
            "xg": xg, "wit": wit, "wht": wht, "brow": brow, "identp": identp,
            "h0t": h0t, "c0": np.ascontiguousarray(c0[sl]),
        })
    return in_maps, P


def _host_post(results, P):
    ys_f = np.concatenate(
        [r["ys"][0].transpose(1, 0, 2).astype(np.float32) for r in results], 0)
    ys_b = np.concatenate(
        [r["ys"][1].transpose(1, 0, 2).astype(np.float32) for r in results], 0)
    out_b = np.take_along_axis(ys_b, P[:, :, None], axis=1)
    return np.concatenate([ys_f, out_b], axis=-1).astype(np.float32)


def kernel(**inputs):
    from concourse.bass_utils import run_bass_kernel_spmd
    in_maps, P = _host_prep(**inputs)
    if "nc" not in _CACHE:
        _CACHE["nc"] = build_kernel()
    nc = _CACHE["nc"]
    res = run_bass_kernel_spmd(nc, in_maps, list(range(NCORES)))
    return _host_post(res.results, P)
